# revision 10
# baseline (speedup 1.0000x reference)
"""Multi-head self-attention (B=8, N=1024, C=768, H=12) on 8 trn2 NeuronCores.

Sharding: data-parallel over batch — core b computes batch element b end to
end; weights are replicated. No collectives.

Per-core dataflow (all matmuls on TensorE, out = lhsT.T @ rhs, contraction on
the partition dim):

  1. qkv^T for Q,K in [c', n] layout:  lhsT = Wqkv^T k-tile, rhs = x^T k-tile.
     One [128,1024] PSUM tile per c'-tile (both 512-halves), bias fused into
     the PSUM->SBUF copy on DVE. DMA is prioritized so the pair-0 tiles
     (t=0 and t=6) land first and attention starts ~8us in.
  2. V in token-major per-head blocks [ones(64) | V_h] (128 cols per head):
     the 64 ones columns make the A@V matmul produce the softmax row-sums
     replicated across 64 partitions, so normalization needs no partition
     broadcast. V bias is skipped on-device: since softmax rows sum to 1, it
     folds into an adjusted proj bias bp' = b_proj + W_proj @ b_qkv[V]
     (host-computed).
  3. Per head h: S^T[m, n] = (K_h^T).T-stationary @ Q_h^T (K = d = 64).
     exp via ScalarE reading PSUM, writing SBUF (scale folded into the K
     projection host-side; max-subtraction skipped — scores are O(1) here and
     softmax is shift-invariant so the result is identical).
  4. AV runs one full head deferred, as two consecutive 8-matmul accumulation
     runs (g=0,1) per head — consecutive same-bank accumulation avoids the
     ~90ns/matmul weight-load handoff penalty that interleaved accumulation
     groups pay. Out rows 0:64 = row-sums (replicated), 64:128 = O_h^T.
  5. normalize: custom-DVE fast reciprocal on the replicated sums (PSUM base
     partition 0), then one tensor_mul into ouT[c, n] stacked across heads.
  6. proj is split: the j=0..4 k-tile contributions run as filler inside
     heads 10-11's slots (PE is otherwise ACT/exp-starved there), staged to
     SBUF with the bias added; only the j=5 contribution + final add + y DMA
     (bf16, spread over 3 DMA queues) remain after the last head.

Scheduling: attention slots are emitted per (head, m-tile): scores + exp,
with the previous head's AV runs, V production (head 0), remaining q/k tiles
(heads 1-9) and partial proj (heads 10-11) drained as PE filler inside the
ACT-bound slots. PSUM: 2x[128,1024] (scores + all filler units, queue-
rotated) + 4x[128,512] AV accumulators.

All matmul inputs are bf16 (fp32 accumulate); y is written bf16 and upcast
on host. Measured end-to-end error vs the fp32 reference ~2.4e-3
scale-relative.
"""

import numpy as np
import ml_dtypes

B, N, C = 8, 1024, 768
H, D = 12, 64
HB = 2 * D  # per-head V block width: [ones(64) | V_h(64)]
N_CORES = 8
P = 128
KT = C // P  # 6 contraction tiles
NT = N // P  # 8 token tiles
NQT = 2 * C // P  # 12 q/k c'-tiles; pair p uses tiles p and 6+p

_CACHE: dict = {}


def _build(cfg: dict):
    import concourse.bass as bass
    import concourse.bacc as bacc
    import concourse.mybir as mybir
    import concourse.tile as tile

    dt = mybir.dt
    f32 = dt.float32
    bf16 = dt.bfloat16

    nc = bacc.Bacc("TRN2", target_bir_lowering=False, debug=False,
                   num_devices=N_CORES)

    xT_d = nc.dram_tensor("xT", [C, N], bf16, kind="ExternalInput")
    wqkvT_d = nc.dram_tensor("wqkvT", [C, 3 * C], bf16, kind="ExternalInput")
    wprojT_d = nc.dram_tensor("wprojT", [C, C], bf16, kind="ExternalInput")
    bqk_d = nc.dram_tensor("bqk", [P, NQT], f32, kind="ExternalInput")
    bp_d = nc.dram_tensor("bp", [1, C], f32, kind="ExternalInput")
    y_d = nc.dram_tensor("y", [N, C], bf16, kind="ExternalOutput")

    with tile.TileContext(nc, pool_alloc_mode="queue") as tc:
        with (
            tc.tile_pool(name="const", bufs=1) as cpool,
            tc.tile_pool(name="et", bufs=cfg["et_bufs"]) as etpool,
            tc.tile_pool(name="work", bufs=2) as workpool,
            tc.tile_pool(name="ps_s", bufs=2, space="PSUM") as ps_s,
            tc.tile_pool(name="ps_av", bufs=4, space="PSUM") as ps_av,
        ):
            # ---- resident loads: 8 batched transfers over 5 queues ----
            # Each dma_start costs ~600ns of trigger time serialized on the
            # issuing engine's queue, so inputs are batched into single
            # multi-dim-AP transfers: the pair-0 q/k weight slices (tensor
            # queue) and the two x halves (sync+vector) land first so the
            # PE can start ~5us in; everything else trickles underneath.
            # pair-0 weight slices: per k-tile, Q cols 0:128 and K cols
            # 768:896 — i.e. cols 0:128 of segments 0 and 1.
            # layout: [Q-slice k=0..5 | K-slice k=0..5], 128 cols each
            wqp0 = cpool.tile([P, KT * 2 * P], bf16, name="wqp0", tag="wqp0")
            src_w3 = wqkvT_d.ap().rearrange("(k p) n -> p k n", p=P)
            nc.sync.dma_start(
                wqp0[:, 0:KT * P].rearrange("p (k c) -> p k c", k=KT),
                src_w3[:, :, 0:P])
            nc.sync.dma_start(
                wqp0[:, KT * P:2 * KT * P].rearrange("p (k c) -> p k c", k=KT),
                src_w3[:, :, C:C + P])
            bqk = cpool.tile([P, NQT], f32, name="bqk", tag="bqk")
            nc.gpsimd.dma_start(bqk[:], bqk_d.ap())
            bp = cpool.tile([1, C], f32, name="bp", tag="bp")
            nc.gpsimd.dma_start(bp[:], bp_d.ap())
            # x^T as one tile, two half transfers on separate queues
            xT1 = cpool.tile([P, KT * N], bf16, name="xT1", tag="xT1")
            src_x = xT_d.ap().rearrange("(k p) n -> p k n", p=P)
            nc.scalar.dma_start(
                xT1[:, 0:3 * N].rearrange("p (k n) -> p k n", k=3),
                src_x[:, 0:3, :])
            nc.sync.dma_start(
                xT1[:, 3 * N:KT * N].rearrange("p (k n) -> p k n", k=3),
                src_x[:, 3:KT, :])
            # full qkv weights: Q/K segments on scalar, V segment on gpsimd
            wq1 = cpool.tile([P, KT * 3 * C], bf16, name="wq1", tag="wq1")
            dst_w = wq1[:].rearrange("p (k n) -> p k n", k=KT)
            nc.scalar.dma_start(dst_w[:, :, 0:2 * C], src_w3[:, :, 0:2 * C])
            nc.gpsimd.dma_start(dst_w[:, :, 2 * C:3 * C],
                                src_w3[:, :, 2 * C:3 * C])
            wp1 = cpool.tile([P, KT * C], bf16, name="wp1", tag="wp1")
            nc.gpsimd.dma_start(
                wp1[:].rearrange("p (k n) -> p k n", k=KT),
                wprojT_d.ap().rearrange("(k p) n -> p k n", p=P))
            bp_b = cpool.tile([P, C], f32, name="bp_b", tag="bp_b")
            nc.gpsimd.partition_broadcast(bp_b[:], bp[:])

            def xT(k):
                return xT1[:, k * N:(k + 1) * N]

            def wq(k):
                return wq1[:, k * 3 * C:(k + 1) * 3 * C]

            def wp(k):
                return wp1[:, k * C:(k + 1) * C]

            # ---- tiles ----
            qkT = [cpool.tile([P, N], bf16, name=f"qkT{t}", tag=f"qkT{t}")
                   for t in range(NQT)]
            v = [cpool.tile([P, H * HB], bf16, name=f"v{nt}", tag=f"v{nt}")
                 for nt in range(NT)]
            ouT = [cpool.tile([P, N], bf16, name=f"ouT{j}", tag=f"ouT{j}")
                   for j in range(KT)]
            ysb = [cpool.tile([P, C], f32, name=f"ysb{nt}", tag=f"ysb{nt}")
                   for nt in range(NT)]

            # ---- filler units (each allocates one ps_s [128,1024] tile) ----
            def qk_unit(t):
                # Q^T or K^T tile t: [c'=128, n=1024], 2x6 accumulation runs.
                # Pair-0 tiles (t=0,6) read the early wqp0 slices.
                def w(k):
                    if t == 0:
                        return wqp0[:, k * P:(k + 1) * P]
                    if t == KT:
                        return wqp0[:, (KT + k) * P:(KT + k + 1) * P]
                    s, c0 = (0, t * P) if t < KT else (1, (t - KT) * P)
                    return wq(k)[:, s * C + c0:s * C + c0 + P]
                pm = ps_s.tile([P, N], f32, name="mm", tag="s")
                for g in range(2):
                    for k in range(KT):
                        nc.tensor.matmul(
                            pm[:, g * 512:(g + 1) * 512],
                            w(k),
                            xT(k)[:, g * 512:(g + 1) * 512],
                            start=(k == 0), stop=(k == KT - 1),
                        )
                nc.vector.tensor_scalar_add(qkT[t][:], pm[:], bqk[:, t:t + 1])

            def v_unit(nt):
                # V for token tile nt: per-head blocks [ones(64) | V_h(64)]
                dst = v[nt][:].rearrange("p (h c) -> p h c", c=HB)
                nc.vector.memset(dst[:, :, 0:D], 1.0)
                pm = ps_s.tile([P, N], f32, name="mm", tag="s")
                for off, width in ((0, 512), (512, 256)):
                    for k in range(KT):
                        nc.tensor.matmul(
                            pm[:, off:off + width],
                            xT(k)[:, nt * P:(nt + 1) * P],
                            wq(k)[:, 2 * C + off:2 * C + off + width],
                            start=(k == 0), stop=(k == KT - 1),
                        )
                src = pm[:, 0:C].rearrange("p (h d) -> p h d", d=D)
                nc.vector.tensor_copy(dst[:, :, D:HB], src[:])

            def proj_partial(nt):
                # y[nt] partial: k-tiles j=0..4, staged to SBUF with bias
                pm = ps_s.tile([P, N], f32, name="mm", tag="s")
                for j in range(KT - 1):
                    for off, width in ((0, 512), (512, 256)):
                        nc.tensor.matmul(
                            pm[:, off:off + width],
                            ouT[j][:, nt * P:(nt + 1) * P],
                            wp(j)[:, off:off + width],
                            start=(j == 0), stop=(j == KT - 2),
                        )
                nc.vector.tensor_add(ysb[nt][:], pm[:, 0:C], bp_b[:])

            # ---- attention pieces ----
            class HeadState:
                def __init__(self, h):
                    self.h = h
                    self.off = D * (h % 2)
                    self.ets = []
                    self.o_ps = None

            def score_exp(st, mt):
                qt = qkT[st.h // 2]
                kt = qkT[NQT // 2 + st.h // 2]
                sp = ps_s.tile([P, N], f32, name="sp", tag="s")
                for g in range(2):
                    nc.tensor.matmul(
                        sp[:, g * 512:(g + 1) * 512],
                        kt[st.off:st.off + D, mt * P:(mt + 1) * P],
                        qt[st.off:st.off + D, g * 512:(g + 1) * 512],
                        start=True, stop=True,
                    )
                et = etpool.tile([P, N], bf16, name="et", tag="et")
                nc.scalar.activation(
                    et[:], sp[:], bass.mybir.ActivationFunctionType.Exp)
                st.ets.append(et)

            def av_run(st, g):
                # one consecutive 8-matmul accumulation run into one bank
                if st.o_ps is None:
                    st.o_ps = [ps_av.tile([P, 512], f32, name="o_ps", tag="av")
                               for _ in range(2)]
                for mt in range(NT):
                    nc.tensor.matmul(
                        st.o_ps[g][:],
                        v[mt][:, st.h * HB:(st.h + 1) * HB],
                        st.ets[mt][:, g * 512:(g + 1) * 512],
                        start=(mt == 0), stop=(mt == NT - 1),
                    )

            def normalize(st):
                # sums are replicated on partitions 0:64 of o_ps; O^T on
                # 64:128. recip reads PSUM at base partition 0 (the custom
                # DVE op mis-reads PSUM only at base partition 64).
                for g in range(2):
                    sl = slice(g * 512, (g + 1) * 512)
                    rb = workpool.tile([D, 512], f32, name="rb", tag="rb")
                    nc.vector.reciprocal_approx_fast(rb[:], st.o_ps[g][0:D, :])
                    nc.vector.tensor_mul(
                        ouT[st.h // 2][st.off:st.off + D, sl],
                        st.o_ps[g][D:P, :], rb[:])

            # ---- filler schedule ----
            # h0: the 8 V units. heads 1-9: the 10 remaining q/k tiles
            # (tile t and 6+t land at head <= 2t so pair t is ready before
            # head 2t consumes it). heads 10-11: the 8 proj partials.
            slot_fill: dict = {}
            for nt in range(NT):
                slot_fill[(0, nt)] = (lambda nt=nt: v_unit(nt))
            # spread so each unit lands before its consumer head (tile t
            # consumed from head 2*(t%6) slot 0) and late heads get filler
            # too (their slots are otherwise ACT-paced with PE idle).
            qk_slots = [(1, 2), (1, 6), (2, 2), (2, 6), (3, 2),
                        (4, 2), (5, 2), (6, 2), (7, 2), (8, 2)]
            qk_units = [1, 7, 2, 8, 3, 9, 4, 10, 5, 11]
            for (hh, ss), t in zip(qk_slots, qk_units):
                slot_fill[(hh, ss)] = (lambda t=t: qk_unit(t))
            # proj partials need ouT[0..4] complete, i.e. normalize(9) done
            # (emitted at head 10 slot 5) — only slots after that qualify.
            # Unit nt's j=4 matmuls come last so the first 8 matmuls cover
            # the normalize DVE latency.
            pp_slots = [(10, 6), (10, 7), (11, 0), (11, 2), (11, 3),
                        (11, 6), (11, 7)]
            for (hh, ss), nt in zip(pp_slots, range(NT)):
                slot_fill[(hh, ss)] = (lambda nt=nt: proj_partial(nt))

            # pair-0 q/k tiles up front — unblocks attention immediately
            qk_unit(0)
            qk_unit(6)

            # ---- main loop: head h's slots run its scores/exp, the previous
            # head's AV runs (slots 1 and 4) and its normalize (slot 5), plus
            # filler. AV one head deferred keeps PE off the exp latency.
            prev = None
            for h in range(H):
                st = HeadState(h)
                for mt in range(NT):
                    score_exp(st, mt)
                    if prev is not None:
                        if mt == 1:
                            av_run(prev, 0)
                        elif mt == 4:
                            av_run(prev, 1)
                        elif mt == 5:
                            normalize(prev)
                    u = slot_fill.pop((h, mt), None)
                    if u is not None:
                        u()
                prev = st
            av_run(prev, 0)
            av_run(prev, 1)
            normalize(prev)
            proj_partial(NT - 1)  # covers normalize(11)'s DVE latency

            # ---- tail: j=5 proj contribution + final add + y DMA ----
            dma_engines = [nc.sync, nc.scalar, nc.gpsimd]
            for nt in range(NT):
                pm = ps_s.tile([P, N], f32, name="mm", tag="s")
                for off, width in ((0, 512), (512, 256)):
                    nc.tensor.matmul(
                        pm[:, off:off + width],
                        ouT[KT - 1][:, nt * P:(nt + 1) * P],
                        wp(KT - 1)[:, off:off + width],
                        start=True, stop=True,
                    )
                yb = workpool.tile([P, C], bf16, name="yb", tag="yb",
                                   bufs=3)
                nc.vector.tensor_add(yb[:], pm[:, 0:C], ysb[nt][:])
                dma_engines[nt % 3].dma_start(
                    y_d.ap()[nt * P:(nt + 1) * P, :], yb[:])

    nc.compile()
    return nc


DEFAULT_CFG = dict(et_bufs=16)


def _host_prep(x, W_qkv, b_qkv, W_proj, b_proj, cfg):
    """Shard + lay out host-side numpy inputs per core."""
    scale = 1.0 / np.sqrt(D)
    wqkvT = np.ascontiguousarray(W_qkv.T).astype(np.float32)
    # fold the 1/sqrt(D) score scale into the K projection (cols C:2C)
    wqkvT[:, C:2 * C] *= scale
    wqkvT = wqkvT.astype(ml_dtypes.bfloat16)
    wprojT = np.ascontiguousarray(W_proj.T).astype(ml_dtypes.bfloat16)
    bqk_f = b_qkv[:2 * C].astype(np.float32).copy()
    bqk_f[C:2 * C] *= scale
    bqk = np.ascontiguousarray(bqk_f.reshape(NQT, P).T).astype(np.float32)
    bp_eff = (b_proj.astype(np.float64)
              + W_proj.astype(np.float64) @ b_qkv[2 * C:].astype(np.float64))
    bp = bp_eff.astype(np.float32).reshape(1, C)
    in_maps = []
    for b in range(N_CORES):
        xT = np.ascontiguousarray(x[b].T).astype(ml_dtypes.bfloat16)
        in_maps.append({"xT": xT, "wqkvT": wqkvT, "wprojT": wprojT,
                        "bqk": bqk, "bp": bp})
    return in_maps


def get_nc(cfg=None):
    cfg = dict(DEFAULT_CFG, **(cfg or {}))
    key = tuple(sorted(cfg.items()))
    if key not in _CACHE:
        _CACHE[key] = _build(cfg)
    return _CACHE[key]


def run(inputs, cfg=None, **run_kwargs):
    from concourse import bass_utils

    cfg = dict(DEFAULT_CFG, **(cfg or {}))
    nc = get_nc(cfg)
    in_maps = _host_prep(inputs["x"], inputs["W_qkv"], inputs["b_qkv"],
                         inputs["W_proj"], inputs["b_proj"], cfg)
    res = bass_utils.run_bass_kernel_spmd(
        nc, in_maps, core_ids=list(range(N_CORES)), **run_kwargs)
    out = np.stack([res.results[b]["y"].astype(np.float32)
                    for b in range(N_CORES)], axis=0)
    return out, res


def kernel(**inputs) -> np.ndarray:
    inputs = {k: np.asarray(v) for k, v in inputs.items()}
    out, _ = run(inputs)
    return out


# revision 11
# speedup vs baseline: 1.1914x; 1.1914x over previous
"""Multi-head self-attention (B=8, N=1024, C=768, H=12) on 8 trn2 NeuronCores.

Sharding: data-parallel over batch — core b computes batch element b end to
end; weights are replicated. No collectives.

Per-core dataflow (all matmuls on TensorE, out = lhsT.T @ rhs, contraction on
the partition dim):

  1. qkv^T for Q,K in [c', n] layout:  lhsT = Wqkv^T k-tile, rhs = x^T k-tile.
     One [128,1024] PSUM tile per c'-tile (both 512-halves), bias fused into
     the PSUM->SBUF copy on DVE. DMA is prioritized so the pair-0 tiles
     (t=0 and t=6) land first and attention starts ~8us in.
  2. V in token-major per-head blocks [ones(64) | V_h] (128 cols per head):
     the 64 ones columns make the A@V matmul produce the softmax row-sums
     replicated across 64 partitions, so normalization needs no partition
     broadcast. V bias is skipped on-device: since softmax rows sum to 1, it
     folds into an adjusted proj bias bp' = b_proj + W_proj @ b_qkv[V]
     (host-computed).
  3. Per head h: S^T[m, n] = (K_h^T).T-stationary @ Q_h^T (K = d = 64).
     exp via ScalarE reading PSUM, writing SBUF (scale folded into the K
     projection host-side; max-subtraction skipped — scores are O(1) here and
     softmax is shift-invariant so the result is identical).
  4. AV runs one full head deferred, as two consecutive 8-matmul accumulation
     runs (g=0,1) per head — consecutive same-bank accumulation avoids the
     ~90ns/matmul weight-load handoff penalty that interleaved accumulation
     groups pay. Out rows 0:64 = row-sums (replicated), 64:128 = O_h^T.
  5. normalize: custom-DVE fast reciprocal on the replicated sums (PSUM base
     partition 0), then one tensor_mul into ouT[c, n] stacked across heads.
  6. proj is split: the j=0..4 k-tile contributions run as filler inside
     heads 10-11's slots (PE is otherwise ACT/exp-starved there), staged to
     SBUF with the bias added; only the j=5 contribution + final add + y DMA
     (bf16, spread over 3 DMA queues) remain after the last head.

Scheduling: attention slots are emitted per (head, m-tile): scores + exp,
with the previous head's AV runs, V production (head 0), remaining q/k tiles
(heads 1-9) and partial proj (heads 10-11) drained as PE filler inside the
ACT-bound slots. PSUM: 2x[128,1024] (scores + all filler units, queue-
rotated) + 4x[128,512] AV accumulators.

All matmul inputs are bf16 (fp32 accumulate); y is written bf16 and upcast
on host. Measured end-to-end error vs the fp32 reference ~2.4e-3
scale-relative.
"""

import numpy as np
import ml_dtypes

B, N, C = 8, 1024, 768
H, D = 12, 64
HB = 2 * D  # per-head V block width: [ones(64) | V_h(64)]
N_CORES = 8
P = 128
KT = C // P  # 6 contraction tiles
NT = N // P  # 8 token tiles
NQT = 2 * C // P  # 12 q/k c'-tiles; pair p uses tiles p and 6+p

_CACHE: dict = {}


def _build(cfg: dict):
    import concourse.bass as bass
    import concourse.bacc as bacc
    import concourse.mybir as mybir
    import concourse.tile as tile

    dt = mybir.dt
    f32 = dt.float32
    bf16 = dt.bfloat16

    nc = bacc.Bacc("TRN2", target_bir_lowering=False, debug=False,
                   num_devices=N_CORES)

    xT_d = nc.dram_tensor("xT", [C, N], bf16, kind="ExternalInput")
    wqkvT_d = nc.dram_tensor("wqkvT", [C, 3 * C], bf16, kind="ExternalInput")
    wprojT_d = nc.dram_tensor("wprojT", [C, C], bf16, kind="ExternalInput")
    bqk_d = nc.dram_tensor("bqk", [P, NQT], f32, kind="ExternalInput")
    bp_d = nc.dram_tensor("bp", [1, C], f32, kind="ExternalInput")
    y_d = nc.dram_tensor("y", [N, C], bf16, kind="ExternalOutput")

    with tile.TileContext(nc, pool_alloc_mode="queue") as tc:
        with (
            tc.tile_pool(name="const", bufs=1) as cpool,
            tc.tile_pool(name="et", bufs=cfg["et_bufs"]) as etpool,
            tc.tile_pool(name="work", bufs=2) as workpool,
            tc.tile_pool(name="ps_s", bufs=2, space="PSUM") as ps_s,
            tc.tile_pool(name="ps_av", bufs=4, space="PSUM") as ps_av,
        ):
            # ---- resident loads: 8 batched transfers over 5 queues ----
            # Each dma_start costs ~600ns of trigger time serialized on the
            # issuing engine's queue, so inputs are batched into single
            # multi-dim-AP transfers: the pair-0 q/k weight slices (tensor
            # queue) and the two x halves (sync+vector) land first so the
            # PE can start ~5us in; everything else trickles underneath.
            # pair-0 weight slices: per k-tile, Q cols 0:128 and K cols
            # 768:896 — i.e. cols 0:128 of segments 0 and 1.
            # layout: [Q-slice k=0..5 | K-slice k=0..5], 128 cols each
            wqp0 = cpool.tile([P, KT * 2 * P], bf16, name="wqp0", tag="wqp0")
            src_w3 = wqkvT_d.ap().rearrange("(k p) n -> p k n", p=P)
            nc.sync.dma_start(
                wqp0[:, 0:KT * P].rearrange("p (k c) -> p k c", k=KT),
                src_w3[:, :, 0:P])
            nc.sync.dma_start(
                wqp0[:, KT * P:2 * KT * P].rearrange("p (k c) -> p k c", k=KT),
                src_w3[:, :, C:C + P])
            bqk = cpool.tile([P, NQT], f32, name="bqk", tag="bqk")
            nc.gpsimd.dma_start(bqk[:], bqk_d.ap())
            bp = cpool.tile([1, C], f32, name="bp", tag="bp")
            nc.gpsimd.dma_start(bp[:], bp_d.ap())
            # x^T as one tile, per-k transfers alternating two queues so
            # tiles land progressively (arrival order 0,3,1,4,2,5)
            xT1 = cpool.tile([P, KT * N], bf16, name="xT1", tag="xT1")
            for k in range(KT):
                eng = nc.sync if k < 3 else nc.scalar
                eng.dma_start(xT1[:, k * N:(k + 1) * N],
                              xT_d.ap()[k * P:(k + 1) * P, :])
            # full qkv weights: Q/K segments on scalar, V segment on gpsimd
            wq1 = cpool.tile([P, KT * 3 * C], bf16, name="wq1", tag="wq1")
            dst_w = wq1[:].rearrange("p (k n) -> p k n", k=KT)
            nc.scalar.dma_start(dst_w[:, :, 0:2 * C], src_w3[:, :, 0:2 * C])
            nc.gpsimd.dma_start(dst_w[:, :, 2 * C:3 * C],
                                src_w3[:, :, 2 * C:3 * C])
            wp1 = cpool.tile([P, KT * C], bf16, name="wp1", tag="wp1")
            nc.gpsimd.dma_start(
                wp1[:].rearrange("p (k n) -> p k n", k=KT),
                wprojT_d.ap().rearrange("(k p) n -> p k n", p=P))
            bp_b = cpool.tile([P, C], f32, name="bp_b", tag="bp_b")
            nc.gpsimd.partition_broadcast(bp_b[:], bp[:])

            def xT(k):
                return xT1[:, k * N:(k + 1) * N]

            def wq(k):
                return wq1[:, k * 3 * C:(k + 1) * 3 * C]

            def wp(k):
                return wp1[:, k * C:(k + 1) * C]

            # ---- tiles ----
            qkT = [cpool.tile([P, N], bf16, name=f"qkT{t}", tag=f"qkT{t}")
                   for t in range(NQT)]
            v = [cpool.tile([P, H * HB], bf16, name=f"v{nt}", tag=f"v{nt}")
                 for nt in range(NT)]
            ouT = [cpool.tile([P, N], bf16, name=f"ouT{j}", tag=f"ouT{j}")
                   for j in range(KT)]
            ysb = [cpool.tile([P, C], f32, name=f"ysb{nt}", tag=f"ysb{nt}")
                   for nt in range(NT)]

            # ---- filler units (each allocates one ps_s [128,1024] tile) ----
            def qk_unit(t):
                # Q^T or K^T tile t: [c'=128, n=1024], 2x6 accumulation runs.
                # Pair-0 tiles (t=0,6) read the early wqp0 slices.
                def w(k):
                    if t == 0:
                        return wqp0[:, k * P:(k + 1) * P]
                    if t == KT:
                        return wqp0[:, (KT + k) * P:(KT + k + 1) * P]
                    s, c0 = (0, t * P) if t < KT else (1, (t - KT) * P)
                    return wq(k)[:, s * C + c0:s * C + c0 + P]
                # k in DMA arrival order for the very first unit
                ks = [0, 3, 1, 4, 2, 5] if t == 0 else list(range(KT))
                pm = ps_s.tile([P, N], f32, name="mm", tag="s")
                for g in range(2):
                    for i, k in enumerate(ks):
                        nc.tensor.matmul(
                            pm[:, g * 512:(g + 1) * 512],
                            w(k),
                            xT(k)[:, g * 512:(g + 1) * 512],
                            start=(i == 0), stop=(i == KT - 1),
                        )
                nc.vector.tensor_scalar_add(qkT[t][:], pm[:], bqk[:, t:t + 1])

            def v_unit(nt):
                # V for token tile nt: per-head blocks [ones(64) | V_h(64)]
                dst = v[nt][:].rearrange("p (h c) -> p h c", c=HB)
                nc.vector.memset(dst[:, :, 0:D], 1.0)
                pm = ps_s.tile([P, N], f32, name="mm", tag="s")
                for off, width in ((0, 512), (512, 256)):
                    for k in range(KT):
                        nc.tensor.matmul(
                            pm[:, off:off + width],
                            xT(k)[:, nt * P:(nt + 1) * P],
                            wq(k)[:, 2 * C + off:2 * C + off + width],
                            start=(k == 0), stop=(k == KT - 1),
                        )
                src = pm[:, 0:C].rearrange("p (h d) -> p h d", d=D)
                nc.vector.tensor_copy(dst[:, :, D:HB], src[:])

            def proj_partial(nt):
                # y[nt] partial: k-tiles j=0..4, staged to SBUF with bias
                pm = ps_s.tile([P, N], f32, name="mm", tag="s")
                for j in range(KT - 1):
                    for off, width in ((0, 512), (512, 256)):
                        nc.tensor.matmul(
                            pm[:, off:off + width],
                            ouT[j][:, nt * P:(nt + 1) * P],
                            wp(j)[:, off:off + width],
                            start=(j == 0), stop=(j == KT - 2),
                        )
                nc.vector.tensor_add(ysb[nt][:], pm[:, 0:C], bp_b[:])

            # ---- attention pieces ----
            class HeadState:
                def __init__(self, h):
                    self.h = h
                    self.off = D * (h % 2)
                    self.ets = []
                    self.o_ps = None

            def score_exp(st, mt):
                qt = qkT[st.h // 2]
                kt = qkT[NQT // 2 + st.h // 2]
                sp = ps_s.tile([P, N], f32, name="sp", tag="s")
                for g in range(2):
                    nc.tensor.matmul(
                        sp[:, g * 512:(g + 1) * 512],
                        kt[st.off:st.off + D, mt * P:(mt + 1) * P],
                        qt[st.off:st.off + D, g * 512:(g + 1) * 512],
                        start=True, stop=True,
                    )
                et = etpool.tile([P, N], bf16, name="et", tag="et")
                nc.scalar.activation(
                    et[:], sp[:], bass.mybir.ActivationFunctionType.Exp)
                st.ets.append(et)

            def av_run(st, g):
                # one consecutive 8-matmul accumulation run into one bank
                if st.o_ps is None:
                    st.o_ps = [ps_av.tile([P, 512], f32, name="o_ps", tag="av")
                               for _ in range(2)]
                for mt in range(NT):
                    nc.tensor.matmul(
                        st.o_ps[g][:],
                        v[mt][:, st.h * HB:(st.h + 1) * HB],
                        st.ets[mt][:, g * 512:(g + 1) * 512],
                        start=(mt == 0), stop=(mt == NT - 1),
                    )

            def normalize(st):
                # sums are replicated on partitions 0:64 of o_ps; O^T on
                # 64:128. recip reads PSUM at base partition 0 (the custom
                # DVE op mis-reads PSUM only at base partition 64).
                for g in range(2):
                    sl = slice(g * 512, (g + 1) * 512)
                    rb = workpool.tile([D, 512], f32, name="rb", tag="rb")
                    nc.vector.reciprocal_approx_fast(rb[:], st.o_ps[g][0:D, :])
                    nc.vector.tensor_mul(
                        ouT[st.h // 2][st.off:st.off + D, sl],
                        st.o_ps[g][D:P, :], rb[:])

            # ---- filler schedule ----
            # h0: the 8 V units. heads 1-9: the 10 remaining q/k tiles
            # (tile t and 6+t land at head <= 2t so pair t is ready before
            # head 2t consumes it). heads 10-11: the 8 proj partials.
            slot_fill: dict = {}
            for nt in range(NT):
                slot_fill[(0, nt)] = (lambda nt=nt: v_unit(nt))
            # spread so each unit lands before its consumer head (tile t
            # consumed from head 2*(t%6) slot 0) and late heads get filler
            # too (their slots are otherwise ACT-paced with PE idle).
            qk_slots = [(1, 2), (1, 6), (2, 2), (2, 6), (3, 2),
                        (4, 2), (5, 2), (6, 2), (7, 2), (8, 2)]
            qk_units = [1, 7, 2, 8, 3, 9, 4, 10, 5, 11]
            for (hh, ss), t in zip(qk_slots, qk_units):
                slot_fill[(hh, ss)] = (lambda t=t: qk_unit(t))
            # proj partials need ouT[0..4] complete, i.e. normalize(9) done
            # (emitted at head 10 slot 5) — only slots after that qualify.
            # Unit nt's j=4 matmuls come last so the first 8 matmuls cover
            # the normalize DVE latency.
            pp_slots = [(10, 6), (10, 7), (11, 0), (11, 2), (11, 3),
                        (11, 6), (11, 7)]
            for (hh, ss), nt in zip(pp_slots, range(NT)):
                slot_fill[(hh, ss)] = (lambda nt=nt: proj_partial(nt))

            # pair-0 q/k tiles up front — unblocks attention immediately
            qk_unit(0)
            qk_unit(6)

            # ---- main loop: head h's slots run its scores/exp, the previous
            # head's AV runs (slots 1 and 4) and its normalize (slot 5), plus
            # filler. AV one head deferred keeps PE off the exp latency.
            prev = None
            for h in range(H):
                st = HeadState(h)
                for mt in range(NT):
                    score_exp(st, mt)
                    if prev is not None:
                        if mt == 1:
                            av_run(prev, 0)
                        elif mt == 4:
                            av_run(prev, 1)
                        elif mt == 5:
                            normalize(prev)
                    u = slot_fill.pop((h, mt), None)
                    if u is not None:
                        u()
                prev = st
            av_run(prev, 0)
            av_run(prev, 1)
            normalize(prev)
            proj_partial(NT - 1)  # covers normalize(11)'s DVE latency

            # ---- tail: j=5 proj contribution + final add + y DMA ----
            dma_engines = [nc.sync, nc.scalar, nc.gpsimd]
            for nt in range(NT):
                pm = ps_s.tile([P, N], f32, name="mm", tag="s")
                for off, width in ((0, 512), (512, 256)):
                    nc.tensor.matmul(
                        pm[:, off:off + width],
                        ouT[KT - 1][:, nt * P:(nt + 1) * P],
                        wp(KT - 1)[:, off:off + width],
                        start=True, stop=True,
                    )
                yb = workpool.tile([P, C], bf16, name="yb", tag="yb",
                                   bufs=3)
                nc.vector.tensor_add(yb[:], pm[:, 0:C], ysb[nt][:])
                dma_engines[nt % 3].dma_start(
                    y_d.ap()[nt * P:(nt + 1) * P, :], yb[:])

    nc.compile()
    return nc


DEFAULT_CFG = dict(et_bufs=16)


def _host_prep(x, W_qkv, b_qkv, W_proj, b_proj, cfg):
    """Shard + lay out host-side numpy inputs per core."""
    scale = 1.0 / np.sqrt(D)
    wqkvT = np.ascontiguousarray(W_qkv.T).astype(np.float32)
    # fold the 1/sqrt(D) score scale into the K projection (cols C:2C)
    wqkvT[:, C:2 * C] *= scale
    wqkvT = wqkvT.astype(ml_dtypes.bfloat16)
    wprojT = np.ascontiguousarray(W_proj.T).astype(ml_dtypes.bfloat16)
    bqk_f = b_qkv[:2 * C].astype(np.float32).copy()
    bqk_f[C:2 * C] *= scale
    bqk = np.ascontiguousarray(bqk_f.reshape(NQT, P).T).astype(np.float32)
    bp_eff = (b_proj.astype(np.float64)
              + W_proj.astype(np.float64) @ b_qkv[2 * C:].astype(np.float64))
    bp = bp_eff.astype(np.float32).reshape(1, C)
    in_maps = []
    for b in range(N_CORES):
        xT = np.ascontiguousarray(x[b].T).astype(ml_dtypes.bfloat16)
        in_maps.append({"xT": xT, "wqkvT": wqkvT, "wprojT": wprojT,
                        "bqk": bqk, "bp": bp})
    return in_maps


def get_nc(cfg=None):
    cfg = dict(DEFAULT_CFG, **(cfg or {}))
    key = tuple(sorted(cfg.items()))
    if key not in _CACHE:
        _CACHE[key] = _build(cfg)
    return _CACHE[key]


def run(inputs, cfg=None, **run_kwargs):
    from concourse import bass_utils

    cfg = dict(DEFAULT_CFG, **(cfg or {}))
    nc = get_nc(cfg)
    in_maps = _host_prep(inputs["x"], inputs["W_qkv"], inputs["b_qkv"],
                         inputs["W_proj"], inputs["b_proj"], cfg)
    res = bass_utils.run_bass_kernel_spmd(
        nc, in_maps, core_ids=list(range(N_CORES)), **run_kwargs)
    out = np.stack([res.results[b]["y"].astype(np.float32)
                    for b in range(N_CORES)], axis=0)
    return out, res


def kernel(**inputs) -> np.ndarray:
    inputs = {k: np.asarray(v) for k, v in inputs.items()}
    out, _ = run(inputs)
    return out
